# revision 25
# baseline (speedup 1.0000x reference)
"""Trainium2 Bass kernel for nn_AutoregressiveGRUWithAttention.

Folded-batch layout: per core 128 batch -> [128 partitions, 64 cols]
(features x batch-lo on partitions 0:64, features x batch-hi on 64:128).
Block-diagonal bf16 weights [128,128]; gate biases as per-partition ACT
bias vectors (no K+1 bias rows).

Scaled split state (all 2x so the GPSIMD mix needs no scalar ops):
  uu    = 2*h(t-1)          (stt on DVE, off-chain)
  rr    = (1+tz)*n          (chain tail, DVE stt)
  pp    = (1-tz)*uu = 2(1-tz)h   (gpsimd mul+sub, off-chain)
  h(t)  = 0.5*rr + 0.25*pp  -> gate matmuls use G_r=G/2 on rr and
                               G_q=G/4 on pp; h never materialized on
                               the chain.
  oB    = 2*o(t-1), attnB = 2*attn  (C_r=C/2 blocks, wfheadr=Wf/2)
Softmax running sums s/acc accumulate in PSUM via identity matmuls.
y outputs staged [64, T*26] (lo|hi packed), un-shuffled host-side.

Chain per step: rr -> pr-close mm -> tanh_r -> t2 -> t3 -> tanh_n -> rr.
"""
import numpy as np
import ml_dtypes

B, L, T, IN, H, OUT = 1024, 64, 128, 13, 64, 13
NCORES, BL, C = 8, 128, 64
BIG = 60.0
BF16 = ml_dtypes.bfloat16
P2 = 2 * H   # 128 partitions

_BLKS = ['HRr', 'HZr', 'HBr', 'FRr', 'FZr', 'FA2r', 'WAr', 'CRr', 'CZr',
         'CAr', 'HRq', 'HZq', 'HBq', 'FRq', 'FZq', 'FA2q', 'I']
_OF = {n: i * P2 for i, n in enumerate(_BLKS)}
_WH_COLS = len(_BLKS) * P2
_XR, _XZ, _XA = 0, P2, 2 * P2
_WX_COLS = 3 * P2
_BRE, _BZE, _BNE, _BRD, _BZD, _BND, _BA, _BB = range(8)

LAST_EXEC_NS = None
TRACE = False
TRACE_DIR = None
WARM_DUMMIES = 0


def _prep_weights(Wih, Whh, bih, bhh, Wf, bf, Wa, ba):
    f8 = np.float64
    Wih, Whh, bih, bhh, Wf, bf, Wa, ba = [np.asarray(a, f8) for a in
                                          (Wih, Whh, bih, bhh, Wf, bf, Wa, ba)]
    Wr, Wz, Wn = Wih[0:H], Wih[H:2 * H], Wih[2 * H:3 * H]
    Ur, Uz, Un = Whh[0:H], Whh[H:2 * H], Whh[2 * H:3 * H]
    br_i, bz_i, bn_i = bih[0:H], bih[H:2 * H], bih[2 * H:3 * H]
    br_h, bz_h, bn_h = bhh[0:H], bhh[H:2 * H], bhh[2 * H:3 * H]

    def blkdiag(G, scale):
        base = scale * G.T
        m = np.zeros((P2, P2), f8)
        m[0:H, 0:H] = base
        m[H:P2, H:P2] = base
        return m

    blk = {}
    blk['HR'] = blkdiag(Ur, 0.5)
    blk['HZ'] = blkdiag(Uz, -0.5)
    blk['HB'] = blkdiag(Un, 0.5)
    blk['CR'] = blkdiag(Wr @ Wf, 0.5)
    blk['CZ'] = blkdiag(Wz @ Wf, -0.5)
    blk['CA'] = blkdiag(Wn @ Wf, 1.0)
    blk['FR'] = blk['HR'] + blk['CR']
    blk['FZ'] = blk['HZ'] + blk['CZ']
    blk['FA2'] = blk['CA'] + blk['HB']
    blk['WA'] = blkdiag(Wa, 1.0)
    for n in ('HR', 'HZ', 'HB', 'FR', 'FZ', 'FA2', 'WA', 'CR', 'CZ', 'CA'):
        blk[n + 'r'] = 0.5 * blk[n]
    for n in ('HR', 'HZ', 'HB', 'FR', 'FZ', 'FA2'):
        blk[n + 'q'] = 0.25 * blk[n]
    blk['I'] = np.eye(P2)

    wh = np.zeros((P2, _WH_COLS), f8)
    for n, off in _OF.items():
        wh[:, off:off + P2] = blk[n]

    def xblk(W, scale):
        base = scale * W.T
        m = np.zeros((2 * IN, P2), f8)
        m[0:IN, 0:H] = base
        m[IN:2 * IN, H:P2] = base
        return m

    wx = np.zeros((2 * IN, _WX_COLS), f8)
    wx[:, _XR:_XR + P2] = xblk(Wr, 0.5)
    wx[:, _XZ:_XZ + P2] = xblk(Wz, -0.5)
    wx[:, _XA:_XA + P2] = xblk(Wn, 1.0)

    mrow2 = np.zeros((2, P2), f8)
    mrow2[0, 0:H] = -0.5 * BIG
    mrow2[1, H:P2] = -0.5 * BIG

    wfheadr = np.zeros((P2, 2 * OUT), f8)
    wfheadr[0:H, 0:OUT] = 0.5 * Wf.T
    wfheadr[H:P2, OUT:2 * OUT] = 0.5 * Wf.T
    bfrow = np.concatenate([bf, bf])[None, :]

    dup = lambda v: np.concatenate([v, v])
    bv = np.zeros((P2, 8), f8)
    bv[:, _BRE] = dup(0.5 * (br_i + br_h))
    bv[:, _BZE] = dup(-0.5 * (bz_i + bz_h))
    bv[:, _BNE] = dup(bn_i + 0.5 * bn_h)
    bv[:, _BRD] = dup(0.5 * (br_i + br_h + Wr @ bf))
    bv[:, _BZD] = dup(-0.5 * (bz_i + bz_h + Wz @ bf))
    bv[:, _BND] = dup(bn_i + Wn @ bf + 0.5 * bn_h)
    bv[:, _BA] = dup(ba)
    bvecrow = dup(0.5 * bn_h)[None, :]   # pb bias matmul: lhsT [1,P2]

    return dict(
        wh=np.ascontiguousarray(wh, BF16),
        wx=np.ascontiguousarray(wx, BF16),
        mrow2=np.ascontiguousarray(mrow2, BF16),
        wfheadr=np.ascontiguousarray(wfheadr, BF16),
        bfrow=np.ascontiguousarray(bfrow, BF16),
        bv=np.ascontiguousarray(bv, np.float32),
        bvecrow=np.ascontiguousarray(bvecrow, BF16),
    )


def _prep_core(x_core, len_core, l_steps=L):
    x_core = np.asarray(x_core, np.float32)
    xT = np.zeros((2 * IN, l_steps, C), np.float32)
    xT[0:IN] = np.transpose(x_core[0:C, 0:l_steps, :], (2, 1, 0))
    xT[IN:2 * IN] = np.transpose(x_core[C:BL, 0:l_steps, :], (2, 1, 0))
    lens = np.asarray(len_core)
    valid = (np.arange(l_steps)[:, None] < lens[None, :])      # [L,128]
    invm = np.zeros((2, l_steps, C), np.float32)
    invm[0] = 1.0 - valid[:, 0:C]
    invm[1] = 1.0 - valid[:, C:BL]
    m63 = np.zeros((P2, C), np.float32)
    m63[0:H] = valid[l_steps - 1, 0:C].astype(np.float32)
    m63[H:P2] = valid[l_steps - 1, C:BL].astype(np.float32)
    ones1 = np.ones((1, C), np.float32)
    return (np.ascontiguousarray(xT.reshape(2 * IN, l_steps * C), BF16),
            np.ascontiguousarray(invm.reshape(2, l_steps * C), BF16),
            m63,
            np.ascontiguousarray(ones1, BF16))


def build_nc(l_steps=L, t_steps=T, compile=True):
    import concourse.bacc as bacc
    import concourse.tile as tile
    from concourse import mybir
    from contextlib import ExitStack

    f32 = mybir.dt.float32
    bf = mybir.dt.bfloat16
    AF = mybir.ActivationFunctionType
    OP = mybir.AluOpType

    nc = bacc.Bacc("TRN2", target_bir_lowering=False, debug=False,
                   num_devices=NCORES)
    d_xT = nc.declare_dram_parameter("xT", [2 * IN, l_steps * C], bf, isOutput=False)
    d_invm = nc.declare_dram_parameter("invm", [2, l_steps * C], bf, isOutput=False)
    d_m63 = nc.declare_dram_parameter("m63", [P2, C], f32, isOutput=False)
    d_wh = nc.declare_dram_parameter("wh", [P2, _WH_COLS], bf, isOutput=False)
    d_wx = nc.declare_dram_parameter("wx", [2 * IN, _WX_COLS], bf, isOutput=False)
    d_mrow2 = nc.declare_dram_parameter("mrow2", [2, P2], bf, isOutput=False)
    d_wfheadr = nc.declare_dram_parameter("wfheadr", [P2, 2 * OUT], bf, isOutput=False)
    d_bfrow = nc.declare_dram_parameter("bfrow", [1, 2 * OUT], bf, isOutput=False)
    d_bv = nc.declare_dram_parameter("bv", [P2, 8], f32, isOutput=False)
    d_bvecrow = nc.declare_dram_parameter("bvecrow", [1, P2], bf, isOutput=False)
    d_ones1 = nc.declare_dram_parameter("ones1", [1, C], bf, isOutput=False)
    d_out = nc.declare_dram_parameter("out", [C, t_steps * 2 * OUT], f32, isOutput=True)

    with tile.TileContext(nc) as tc, ExitStack() as ctx:
        const = ctx.enter_context(tc.tile_pool(name="const", bufs=1))
        temps = ctx.enter_context(tc.tile_pool(name="temps", bufs=3))
        p_g = ctx.enter_context(tc.tile_pool(name="p_g", bufs=3, space="PSUM"))
        p_l1 = ctx.enter_context(tc.tile_pool(name="p_l1", bufs=2, space="PSUM"))
        p_l2 = ctx.enter_context(tc.tile_pool(name="p_l2", bufs=2, space="PSUM"))

        xT = const.tile([2 * IN, l_steps * C], bf)
        invm = const.tile([2, l_steps * C], bf)
        m63 = const.tile([P2, C], f32)
        wh = const.tile([P2, _WH_COLS], bf)
        wx = const.tile([2 * IN, _WX_COLS], bf)
        mrow2 = const.tile([2, P2], bf)
        wfheadr = const.tile([P2, 2 * OUT], bf)
        bfrow = const.tile([1, 2 * OUT], bf)
        bv = const.tile([P2, 8], f32)
        bvecrow = const.tile([1, P2], bf)
        ones1 = const.tile([1, C], bf)

        rr = const.tile([P2, C], bf)
        pp = const.tile([P2, C], bf)
        uu = const.tile([P2, C], bf)
        oB = const.tile([P2, C], bf)
        attnB = const.tile([P2, C], bf)
        e_t = const.tile([P2, C], bf)
        eo_t = const.tile([P2, C], bf)
        rec = const.tile([P2, C], f32)
        s_sb = const.tile([P2, C], f32)
        acc_sb = const.tile([P2, C], f32)
        ystage = const.tile([C, t_steps * 2 * OUT], f32)

        for dst, src in ((xT, d_xT), (invm, d_invm), (m63, d_m63),
                         (wh, d_wh), (wx, d_wx), (mrow2, d_mrow2),
                         (wfheadr, d_wfheadr), (bfrow, d_bfrow), (bv, d_bv),
                         (bvecrow, d_bvecrow), (ones1, d_ones1)):
            nc.sync.dma_start(out=dst, in_=src[:])

        nc.vector.memset(rr, 0.0)
        nc.vector.memset(pp, 0.0)
        nc.vector.memset(s_sb, 0.0)
        nc.vector.memset(acc_sb, 0.0)

        W = lambda name: wh[:, _OF[name]:_OF[name] + P2]
        bvc = lambda col: bv[:, col:col + 1]
        mm = nc.tensor.matmul

        def gate_mms(r2, hside, mask_rhs=None, xcol=None):
            """4 gate psum groups.  r2: extra rhs (attnB/oB, 2x scale, via
            C_r blocks); hside: (pp-weight-names, rr-weight-names)."""
            (wpr, wpz, wpb, wpa), (wrr, wrz, wrb, wra) = hside
            pg = p_g.tile([P2, 4 * C], f32, tag="pg")
            pr = pg[:, 0:C]
            pz = pg[:, C:2 * C]
            pa = pg[:, 2 * C:3 * C]
            pb = pg[:, 3 * C:4 * C]
            xs = xT[:, xcol] if xcol is not None else None
            # r group first: tanh_r is the chain head.
            if xs is not None:
                mm(pr, wx[:, _XR:_XR + P2], xs, start=True, stop=False)
            if r2 is not None:
                mm(pr, W('CRr'), r2[:], start=(xs is None), stop=False)
            mm(pr, W(wpr), pp[:], start=(xs is None and r2 is None), stop=False)
            mm(pr, W(wrr), rr[:], start=False, stop=True)
            # z group
            if xs is not None:
                mm(pz, wx[:, _XZ:_XZ + P2], xs, start=True, stop=False)
            if r2 is not None:
                mm(pz, W('CZr'), r2[:], start=(xs is None), stop=False)
            if mask_rhs is not None:
                mm(pz, mrow2[:], mask_rhs, start=False, stop=False)
            mm(pz, W(wpz), pp[:], start=(xs is None and r2 is None), stop=False)
            mm(pz, W(wrz), rr[:], start=False, stop=True)
            # b group (pb bias via K=1 matmul)
            mm(pb, bvecrow[:], ones1[:], start=True, stop=False)
            mm(pb, W(wpb), pp[:], start=False, stop=False)
            mm(pb, W(wrb), rr[:], start=False, stop=True)
            # a group
            if xs is not None:
                mm(pa, wx[:, _XA:_XA + P2], xs, start=True, stop=False)
            if r2 is not None:
                mm(pa, W('CAr'), r2[:], start=(xs is None), stop=False)
            mm(pa, W(wpa), pp[:], start=(xs is None and r2 is None), stop=False)
            mm(pa, W(wra), rr[:], start=False, stop=True)
            return pr, pz, pa, pb

        def chain_rz(pr, pz, b_r, b_z):
            tr = temps.tile([P2, C], bf, tag="tr")
            nc.scalar.activation(out=tr, in_=pr, func=AF.Tanh, bias=bvc(b_r))
            tz = temps.tile([P2, C], bf, tag="tz")
            nc.scalar.activation(out=tz, in_=pz, func=AF.Tanh, bias=bvc(b_z))
            return tr, tz

        def chain_n(tr, pb, pa, b_n):
            t2 = temps.tile([P2, C], f32, tag="t2")
            nc.vector.tensor_mul(out=t2, in0=pb, in1=tr)
            nc.vector.tensor_add(out=pb, in0=t2, in1=pa)
            n = temps.tile([P2, C], bf, tag="n")
            nc.scalar.activation(out=n, in_=pb, func=AF.Tanh, bias=bvc(b_n))
            return n

        def upd_uu():
            # uu = 2*h(t-1) = 0.5*pp + rr
            nc.vector.scalar_tensor_tensor(out=uu[:], in0=pp[:], scalar=0.5,
                                           in1=rr[:], op0=OP.mult, op1=OP.add)

        def mix_tail(tz, n, engine):
            """pp = (1-tz)*uu on `engine` (no scalar ops -> gpsimd-safe);
            rr = (1+tz)*n on DVE (chain tail)."""
            p1 = temps.tile([P2, C], f32, tag="p1")
            engine.tensor_mul(out=p1, in0=tz, in1=uu[:])
            engine.tensor_sub(out=pp[:], in0=uu[:], in1=p1)
            nc.vector.scalar_tensor_tensor(out=rr[:], in0=tz, scalar=1.0,
                                           in1=n, op0=OP.add, op1=OP.mult)

        ENC_H = (('HRq', 'HZq', 'HBq', 'HBq'), ('HRr', 'HZr', 'HBr', 'HBr'))
        DEC_H = (('FRq', 'FZq', 'HBq', 'FA2q'), ('FRr', 'FZr', 'HBr', 'FA2r'))

        # ================= encoder =================
        for t in range(l_steps):
            xcol = slice(t * C, (t + 1) * C)
            mask_rhs = invm[:, xcol] if t < l_steps - 1 else None
            pr, pz, pa, pb = gate_mms(None, ENC_H, mask_rhs, xcol)
            if t > 0:
                upd_uu()
            else:
                nc.vector.memset(uu, 0.0)
            tr, tz = chain_rz(pr, pz, _BRE, _BZE)
            n = chain_n(tr, pb, pa, _BNE)
            if t < l_steps - 1:
                mix_tail(tz, n, nc.vector)
            else:
                # d = n - 0.5*uu; tzd = (tz+1)*d; oB = (tzd+uu)*m63 (=2*out63)
                d = temps.tile([P2, C], f32, tag="p1")
                nc.vector.scalar_tensor_tensor(out=d, in0=uu[:], scalar=-0.5,
                                               in1=n, op0=OP.mult, op1=OP.add)
                tzd = temps.tile([P2, C], f32, tag="t2")
                nc.vector.scalar_tensor_tensor(out=tzd, in0=tz, scalar=1.0,
                                               in1=d, op0=OP.add, op1=OP.mult)
                q1 = temps.tile([P2, C], f32, tag="tr")
                nc.vector.tensor_add(out=q1, in0=tzd, in1=uu[:])
                nc.vector.tensor_mul(out=oB[:], in0=q1, in1=m63)
                nc.vector.tensor_mul(out=rr[:], in0=m63, in1=tzd)
                # pp = 2*uu so that G_q@pp = G_r@uu = G@h62
                nc.vector.scalar_tensor_tensor(out=pp[:], in0=uu[:], scalar=1.0,
                                               in1=uu[:], op0=OP.mult, op1=OP.add)

        # ================= decoder =================
        for t in range(t_steps):
            if t == 0:
                pr, pz, pa, pb = gate_mms(oB, ENC_H)
            elif t == 1:
                pr, pz, pa, pb = gate_mms(None, DEC_H)
            else:
                pr, pz, pa, pb = gate_mms(attnB, DEC_H)
            # deferred attention/output emission for step t-1 on PE:
            if t >= 1:
                ly1 = p_l1.tile([P2, C], f32, tag="ly1")
                ly2f = p_l2.tile([C, 2 * OUT], f32, tag="ly2")
                ly2 = ly2f[:]
                if t >= 2:
                    mm(ly1, W('WAr'), attnB[:], start=True, stop=False)
            upd_uu()
            if t >= 1:
                mm(ly1, W('WAr'), uu[:], start=(t == 1), stop=True)
                mm(ly2, ones1[:], bfrow[:], start=True, stop=False)
                if t >= 2:
                    mm(ly2, attnB[:], wfheadr[:], start=False, stop=False)
                mm(ly2, uu[:], wfheadr[:], start=False, stop=True)
            tr, tz = chain_rz(pr, pz, _BRD, _BZD)
            # exp for step t-1 sits between tz and n in the ACT queue
            if t >= 1:
                nc.scalar.activation(out=e_t[:], in_=ly1, func=AF.Exp,
                                     bias=bvc(_BA))
                if t >= 2:
                    nc.gpsimd.tensor_add(out=oB[:], in0=uu[:], in1=attnB[:])
                else:
                    nc.vector.tensor_copy(out=oB[:], in_=uu[:])
                nc.gpsimd.tensor_mul(out=eo_t[:], in0=e_t[:], in1=oB[:])
                nc.gpsimd.tensor_add(out=s_sb[:], in0=s_sb[:], in1=e_t[:])
                nc.vector.tensor_add(out=acc_sb[:], in0=acc_sb[:], in1=eo_t[:])
            n = chain_n(tr, pb, pa, _BND)
            if t >= 1:
                nc.scalar.copy(out=ystage[:, (t - 1) * 2 * OUT:t * 2 * OUT],
                               in_=ly2)
            mix_tail(tz, n, nc.gpsimd if t >= 1 else nc.vector)
            if t >= 1:
                nc.vector.reciprocal_approx_fast(out=rec, in_=s_sb[:])
                nc.vector.tensor_mul(out=attnB[:], in0=acc_sb[:], in1=rec)

        # final emission (t_steps-1): only y output needed
        upd_uu()
        ly2f = p_l2.tile([C, 2 * OUT], f32, tag="ly2")
        ly2 = ly2f[:]
        mm(ly2, ones1[:], bfrow[:], start=True, stop=False)
        mm(ly2, attnB[:], wfheadr[:], start=False, stop=False)
        mm(ly2, uu[:], wfheadr[:], start=False, stop=True)
        nc.scalar.copy(out=ystage[:, (t_steps - 1) * 2 * OUT:t_steps * 2 * OUT],
                       in_=ly2)

        nc.sync.dma_start(out=d_out[:], in_=ystage)
    if compile:
        nc.compile()
    return nc


def _make_in_maps(inputs, l_steps=L, t_steps=T):
    x = np.asarray(inputs["x"], np.float32)
    lengths = np.asarray(inputs["lengths"])
    w = _prep_weights(inputs["Wih"], inputs["Whh"], inputs["bih"],
                      inputs["bhh"], inputs["Wf"], inputs["bf"],
                      inputs["Wa"], inputs["ba"])
    in_maps = []
    for c in range(NCORES):
        sl = slice(c * BL, (c + 1) * BL)
        xT, invm, m63, ones1 = _prep_core(x[sl], lengths[sl], l_steps)
        in_maps.append(dict(xT=xT, invm=invm, m63=m63, ones1=ones1, **w))
    return in_maps


def kernel(**inputs):
    global LAST_EXEC_NS, TRACE_DIR
    from concourse.bass_utils import run_bass_kernel_spmd
    t_steps = int(inputs.get("output_length", T))
    assert t_steps == T, f"hardcoded for output_length={T}, got {t_steps}"
    nc = build_nc()
    in_maps = _make_in_maps(inputs)
    kw = {}
    if TRACE:
        import tempfile
        TRACE_DIR = tempfile.mkdtemp(prefix="bass_trace_")
        kw = dict(trace=True, tmpdir=TRACE_DIR)
    res = None
    for attempt in range(3):
        try:
            res = run_bass_kernel_spmd(nc, in_maps, list(range(NCORES)), **kw)
            break
        except Exception:
            if attempt == 2:
                raise
    LAST_EXEC_NS = res.exec_time_ns
    outs = []
    for c in range(NCORES):
        y = np.asarray(res.results[c]["out"]).reshape(C, T, 2, OUT)
        full = np.concatenate([y[:, :, 0, :], y[:, :, 1, :]], axis=0)
        outs.append(full)                              # [128, T, OUT]
    return np.concatenate(outs, axis=0)
